# revision 1
# baseline (speedup 1.0000x reference)
import numpy as np
import ml_dtypes

import concourse.bass as bass
import concourse.bacc as bacc
import concourse.mybir as mybir
from concourse.tile import TileContext
from concourse.bass_utils import run_bass_kernel_spmd

BF16 = ml_dtypes.bfloat16
F32 = np.float32

B, H, W, D, K = 4, 384, 384, 16, 32
NCORES = 8
NPIX_TOT = B * H * W
NPIX = NPIX_TOT // NCORES
P = 128
TC = NPIX // P
TCP = 588
NG = TCP // 7
GW = 7 * 17
WCOLS = GW * (NG - 1) + 128
WCOLS_PAD = 10016
NB = TC // 8
QS = TC // 4
LAB_PAD = 100.0

PUSH_MARGIN = 0.25
PUSH_W = 1.0
PULL_W = 0.1
NCMP = K * (K - 1) / 2.0

_built = {}


def _build_launch_a():
    nc = bacc.Bacc("TRN2", target_bir_lowering=False, debug=False)
    bf = mybir.dt.bfloat16
    f32 = mybir.dt.float32

    emb17 = nc.dram_tensor("emb17", [P, WCOLS_PAD], bf, kind="ExternalInput")
    labels = nc.dram_tensor("labels", [P, TCP], bf, kind="ExternalInput")
    outA = nc.dram_tensor("outA", [P, 224], f32, kind="ExternalOutput")

    with TileContext(nc) as tc:
        with (
            tc.tile_pool(name="sbuf", bufs=1) as pool,
            tc.tile_pool(name="psum", bufs=1, space="PSUM") as psum_pool,
        ):
            emb_sb = pool.tile([P, WCOLS_PAD], bf)
            lab_sb = pool.tile([P, TCP], bf)
            onehot = pool.tile([P, K, TCP], bf)

            nc.sync.dma_start(out=lab_sb[:], in_=labels.ap())
            NCH = 4
            ch = WCOLS_PAD // NCH
            for i in range(NCH):
                nc.sync.dma_start(
                    out=emb_sb[:, i * ch : (i + 1) * ch],
                    in_=emb17.ap()[:, i * ch : (i + 1) * ch],
                )

            half = TCP // 2
            for h in range(2):
                sl = slice(h * half, (h + 1) * half)
                for k in range(K):
                    nc.vector.tensor_scalar(
                        out=onehot[:, k, sl],
                        in0=lab_sb[:, sl],
                        scalar1=float(k),
                        scalar2=None,
                        op0=mybir.AluOpType.is_equal,
                    )

            ps = psum_pool.tile([P, 7, K], mybir.dt.float32)
            for g in range(NG):
                nc.tensor.matmul(
                    ps[:],
                    emb_sb[:, GW * g : GW * g + 128],
                    onehot[:, :, 7 * g : 7 * g + 7].rearrange("p k t -> p t k"),
                    start=(g == 0),
                    stop=(g == NG - 1),
                )

            evac = pool.tile([P, 7 * K], f32)
            nc.vector.tensor_copy(out=evac[:], in_=ps[:].rearrange("p a b -> p (a b)"))
            nc.sync.dma_start(out=outA.ap(), in_=evac[:])
    nc.compile()
    return nc


def _build_launch_b():
    nc = bacc.Bacc("TRN2", target_bir_lowering=False, debug=False)
    bf = mybir.dt.bfloat16
    f32 = mybir.dt.float32

    emb16 = nc.dram_tensor("emb16", [P, TC * D], bf, kind="ExternalInput")
    lab4 = nc.dram_tensor("lab4", [P, QS * P], bf, kind="ExternalInput")
    iotaT = nc.dram_tensor("iotaT", [P, 1], f32, kind="ExternalInput")
    centsT = nc.dram_tensor("centsT", [P, 64], bf, kind="ExternalInput")
    cpp = nc.dram_tensor("cpp", [P, D], bf, kind="ExternalInput")
    cjd = nc.dram_tensor("cjd", [P, K * D], bf, kind="ExternalInput")
    triu = nc.dram_tensor("triu", [P, K], bf, kind="ExternalInput")
    pacc_d = nc.dram_tensor("pacc", [P, 4], f32, kind="ExternalOutput")
    pushp_d = nc.dram_tensor("pushp", [P, 1], f32, kind="ExternalOutput")

    with TileContext(nc) as tc:
        with (
            tc.tile_pool(name="sbuf", bufs=1) as pool,
            tc.tile_pool(name="work", bufs=3) as wpool,
            tc.tile_pool(name="psum", bufs=2, space="PSUM") as psum_pool,
        ):
            emb_sb = pool.tile([P, TC, D], bf)
            lab4_sb = pool.tile([P, QS * P], bf)
            iotaT_sb = pool.tile([P, 1], f32)
            centsT_sb = pool.tile([P, 64], bf)
            cpp_sb = pool.tile([P, D], bf)
            cjd_sb = pool.tile([P, K, D], bf)
            triu_sb = pool.tile([P, K], bf)
            oh4 = pool.tile([P, QS * P], bf)
            dist = pool.tile([P, TC, 4], bf)
            pacc = pool.tile([P, 4], f32)
            pushp = pool.tile([P, 1], f32)

            nc.sync.dma_start(out=iotaT_sb[:], in_=iotaT.ap())
            nc.sync.dma_start(out=centsT_sb[:], in_=centsT.ap())
            nc.sync.dma_start(out=cpp_sb[:], in_=cpp.ap())
            nc.sync.dma_start(out=cjd_sb[:], in_=cjd.ap().rearrange("p (a b) -> p a b", b=D))
            nc.sync.dma_start(out=triu_sb[:], in_=triu.ap())

            NCH = 4
            ech = TC // NCH
            for i in range(NCH):
                nc.sync.dma_start(
                    out=emb_sb[:, i * ech : (i + 1) * ech, :],
                    in_=emb16.ap().rearrange("p (t d) -> p t d", d=D)[
                        :, i * ech : (i + 1) * ech, :
                    ],
                )
            lch = (QS * P) // NCH
            for i in range(NCH):
                nc.sync.dma_start(
                    out=lab4_sb[:, i * lch : (i + 1) * lch],
                    in_=lab4.ap()[:, i * lch : (i + 1) * lch],
                )

            NOH = 12
            oc = (QS * P) // NOH
            for i in range(NOH):
                sl = slice(i * oc, (i + 1) * oc)
                nc.vector.tensor_scalar(
                    out=oh4[:, sl],
                    in0=lab4_sb[:, sl],
                    scalar1=iotaT_sb[:, 0:1],
                    scalar2=None,
                    op0=mybir.AluOpType.is_equal,
                )

            NSB = TC // 32
            for sb in range(NSB):
                t0 = 32 * sb
                pss = [
                    psum_pool.tile(
                        [P, 8, 4, D], mybir.dt.float32, tag=f"ps{s}",
                        name=f"ps{s}_{sb}",
                    )
                    for s in range(4)
                ]
                for j in range(8):
                    for s in range(4):
                        q = 8 * sb + j
                        nc.tensor.matmul(
                            pss[s][:, j, :, :].rearrange("p a b -> p (a b)"),
                            oh4[32 * s : 32 * s + 32, P * q : P * (q + 1)],
                            centsT_sb[32 * s : 32 * s + 32, :],
                            start=True,
                            stop=True,
                            tile_position=(32 * s, 0),
                        )
                for s in range(4):
                    gev = wpool.tile([P, 8, 4, D], bf, tag=f"gev{s}")
                    nc.scalar.copy(out=gev[:], in_=pss[s][:])
                    diff = wpool.tile([P, 8, 4, D], bf, tag=f"diff{s}")
                    nc.vector.tensor_tensor(
                        out=diff[:],
                        in0=gev[:],
                        in1=emb_sb[:, t0 + s : t0 + s + 29 : 4, :]
                        .unsqueeze(2)
                        .broadcast_to([P, 8, 4, D]),
                        op=mybir.AluOpType.subtract,
                    )
                    with nc.allow_low_precision("dist in bf16; error averages out"):
                        nc.vector.tensor_reduce(
                            out=dist[:, t0 + s : t0 + s + 29 : 4, :],
                            in_=diff[:],
                            axis=mybir.AxisListType.X,
                            op=mybir.AluOpType.add,
                            apply_absolute_value=True,
                        )

            sq = pool.tile([P, TC, 4], f32)
            nc.vector.tensor_tensor(
                out=sq[:], in0=dist[:], in1=dist[:], op=mybir.AluOpType.mult
            )
            nc.vector.tensor_reduce(
                out=pacc[:],
                in_=sq[:].rearrange("p t b -> p b t"),
                axis=mybir.AxisListType.X,
                op=mybir.AluOpType.add,
            )
            nc.sync.dma_start(out=pacc_d.ap(), in_=pacc[:])

            pd_diff = pool.tile([P, K, D], bf)
            nc.vector.tensor_tensor(
                out=pd_diff[:],
                in0=cpp_sb[:].unsqueeze(1).broadcast_to([P, K, D]),
                in1=cjd_sb[:],
                op=mybir.AluOpType.subtract,
            )
            pd = pool.tile([P, K], f32)
            nc.vector.tensor_reduce(
                out=pd[:],
                in_=pd_diff[:],
                axis=mybir.AxisListType.X,
                op=mybir.AluOpType.add,
                apply_absolute_value=True,
            )
            m = pool.tile([P, K], f32)
            nc.vector.tensor_scalar(
                out=m[:],
                in0=pd[:],
                scalar1=PUSH_MARGIN,
                scalar2=0.0,
                op0=mybir.AluOpType.subtract,
                op1=mybir.AluOpType.min,
            )
            msq = pool.tile([P, K], f32)
            nc.vector.tensor_tensor(
                out=msq[:], in0=m[:], in1=m[:], op=mybir.AluOpType.mult
            )
            msqm = pool.tile([P, K], f32)
            nc.vector.tensor_tensor(
                out=msqm[:], in0=msq[:], in1=triu_sb[:], op=mybir.AluOpType.mult
            )
            nc.vector.tensor_reduce(
                out=pushp[:],
                in_=msqm[:],
                axis=mybir.AxisListType.X,
                op=mybir.AluOpType.add,
            )
            nc.sync.dma_start(out=pushp_d.ap(), in_=pushp[:])
    nc.compile()
    return nc


def _get(name):
    if name not in _built:
        if name == "A":
            _built[name] = _build_launch_a()
        else:
            _built[name] = _build_launch_b()
    return _built[name]


def _prep_a(emb_flat, lab_flat):
    in_maps = []
    for c in range(NCORES):
        e = emb_flat[c * NPIX : (c + 1) * NPIX].astype(BF16).reshape(P, TC, D)
        l = lab_flat[c * NPIX : (c + 1) * NPIX].reshape(P, TC)
        e17 = np.zeros((P, TCP, 17), dtype=BF16)
        e17[:, :TC, :D] = e
        e17[:, :, D] = BF16(1.0)
        w = np.zeros((P, WCOLS_PAD), dtype=BF16)
        w[:, : TCP * 17] = e17.reshape(P, TCP * 17)
        lb = np.full((P, TCP), LAB_PAD, dtype=BF16)
        lb[:, :TC] = l.astype(BF16)
        in_maps.append({"emb17": w, "labels": lb})
    return in_maps


def _reduce_a(results):
    sums = np.zeros((B, K, D), dtype=np.float64)
    cnts = np.zeros((B, K), dtype=np.float64)
    for c in range(NCORES):
        o = results[c]["outA"].astype(np.float64).reshape(P, 7, K)
        s = c // 2
        for j in range(7):
            blk = o[17 * j : 17 * j + 17, j, :]
            sums[s] += blk[:D].T
            cnts[s] += blk[D]
    cents = sums / np.maximum(cnts, 1.0)[:, :, None]
    cents = np.where(cnts[:, :, None] > 0, cents, 0.0)
    return cents, cnts


def _prep_b(emb_flat, lab_flat, cents):
    iotaT = (np.arange(P, dtype=F32) % K).astype(F32).reshape(P, 1)
    centsT = np.zeros((P, 64), dtype=BF16)
    cb = cents.astype(F32)
    for s in range(4):
        centsT[32 * s : 32 * s + 32, :] = (
            cb.transpose(1, 0, 2).reshape(K, 64).astype(BF16)
        )
    cpp = cb.reshape(P, D).astype(BF16)
    cjd = np.zeros((P, K * D), dtype=BF16)
    for b in range(4):
        cjd[32 * b : 32 * b + 32, :] = np.broadcast_to(
            cb[b].reshape(1, K * D), (K, K * D)
        ).astype(BF16)
    triu = np.zeros((P, K), dtype=BF16)
    kk = np.arange(K)
    for b in range(4):
        triu[32 * b : 32 * b + 32, :] = (kk[None, :] > kk[:, None]).astype(BF16)

    in_maps = []
    for c in range(NCORES):
        e = emb_flat[c * NPIX : (c + 1) * NPIX].astype(BF16).reshape(P, TC, D)
        l = lab_flat[c * NPIX : (c + 1) * NPIX].reshape(P, TC)
        lab4 = np.zeros((P, QS * P), dtype=BF16)
        for s in range(4):
            a = l[:, s::4]
            lab4[32 * s : 32 * s + 32, :] = np.broadcast_to(
                a.T.reshape(1, QS * P), (K, QS * P)
            ).astype(BF16)
        in_maps.append(
            {
                "emb16": e.reshape(P, TC * D),
                "lab4": lab4,
                "iotaT": iotaT.copy(),
                "centsT": centsT.copy(),
                "cpp": cpp.copy(),
                "cjd": cjd.copy(),
                "triu": triu.copy(),
            }
        )
    return in_maps


def run_launches(embeddings, labels, trace=False, trace_kwargs=None):
    emb_flat = np.ascontiguousarray(np.asarray(embeddings), dtype=F32).reshape(
        NPIX_TOT, D
    )
    lab_flat = np.ascontiguousarray(np.asarray(labels), dtype=np.int32).reshape(
        NPIX_TOT
    )
    core_ids = list(range(NCORES))

    kwA = dict(trace=trace, **(trace_kwargs or {}))
    resA = run_bass_kernel_spmd(_get("A"), _prep_a(emb_flat, lab_flat), core_ids, **kwA)
    cents, _ = _reduce_a(resA.results)

    resB = run_bass_kernel_spmd(
        _get("B"), _prep_b(emb_flat, lab_flat, cents), core_ids, **kwA
    )
    pull = np.zeros(4, dtype=np.float64)
    for c in range(NCORES):
        pull += resB.results[c]["pacc"].astype(np.float64).sum(axis=0)
    pull /= NPIX_TOT

    pushp = resB.results[0]["pushp"].astype(np.float64).reshape(4, K).sum(axis=1)
    push = pushp / NCMP

    loss = np.mean(PUSH_W * push + PULL_W * pull)
    return np.array(loss, dtype=F32), resA, resB


def kernel(embeddings, labels):
    loss, _, _ = run_launches(embeddings, labels, trace=False)
    return loss



# revision 9
# speedup vs baseline: 1.3801x; 1.3801x over previous
import numpy as np
import ml_dtypes

import concourse.bass as bass
import concourse.bacc as bacc
import concourse.mybir as mybir
from concourse.tile import TileContext
from concourse.bass_utils import run_bass_kernel_spmd

BF16 = ml_dtypes.bfloat16
F32 = np.float32

B, H, W, D, K = 4, 384, 384, 16, 32
NCORES = 8
NPIX_TOT = B * H * W
NPIX = NPIX_TOT // NCORES
P = 128
TC = NPIX // P
TCP = 588
NG = TCP // 7
GW = 7 * 17
WCOLS_PAD = 10016
NBANKS_A = 8
LAB_PAD = 100.0

HT = TC // 2
NW = HT // 8
CR = 48

PUSH_MARGIN = 0.25
PUSH_W = 1.0
PULL_W = 0.1
NCMP = K * (K - 1) / 2.0

CONSUMER_PATTERN = "AADADAADADAADADAAD"
WT = 16

_built = {}


def _build_launch_a():
    nc = bacc.Bacc("TRN2", target_bir_lowering=False, debug=False)
    bf = mybir.dt.bfloat16
    f32 = mybir.dt.float32

    emb17 = nc.dram_tensor("emb17", [P, WCOLS_PAD], bf, kind="ExternalInput")
    labels = nc.dram_tensor("labels", [P, TCP], bf, kind="ExternalInput")
    outA = nc.dram_tensor("outA", [P, NBANKS_A, 7 * K], f32, kind="ExternalOutput")

    with TileContext(nc) as tc:
        with (
            tc.tile_pool(name="sbuf", bufs=1) as pool,
            tc.tile_pool(name="psum", bufs=1, space="PSUM") as psum_pool,
        ):
            emb_sb = pool.tile([P, WCOLS_PAD], bf)
            lab_sb = pool.tile([P, TCP], bf)
            onehot = pool.tile([P, K, TCP], bf)

            nc.sync.dma_start(out=lab_sb[:], in_=labels.ap())
            NCH = 8
            ch = WCOLS_PAD // NCH
            for i in range(NCH):
                nc.sync.dma_start(
                    out=emb_sb[:, i * ch : (i + 1) * ch],
                    in_=emb17.ap()[:, i * ch : (i + 1) * ch],
                )

            half = TCP // 2
            for h in range(2):
                sl = slice(h * half, (h + 1) * half)
                for k in range(K):
                    nc.vector.tensor_scalar(
                        out=onehot[:, k, sl],
                        in0=lab_sb[:, sl],
                        scalar1=float(k),
                        scalar2=None,
                        op0=mybir.AluOpType.is_equal,
                    )

            banks = [
                psum_pool.tile([P, 7, K], mybir.dt.float32, name=f"acc{b}")
                for b in range(NBANKS_A)
            ]
            for g in range(NG):
                nc.tensor.matmul(
                    banks[g % NBANKS_A][:],
                    emb_sb[:, GW * g : GW * g + 128],
                    onehot[:, :, 7 * g : 7 * g + 7].rearrange("p k t -> p t k"),
                    start=(g < NBANKS_A),
                    stop=(g >= NG - NBANKS_A),
                )

            evac = pool.tile([P, NBANKS_A, 7 * K], f32)
            for b in range(NBANKS_A):
                eng = nc.scalar if b % 2 == 0 else nc.vector
                if b % 2 == 0:
                    nc.scalar.copy(
                        out=evac[:, b, :],
                        in_=banks[b][:].rearrange("p a b -> p (a b)"),
                    )
                else:
                    nc.vector.tensor_copy(
                        out=evac[:, b, :],
                        in_=banks[b][:].rearrange("p a b -> p (a b)"),
                    )
            nc.sync.dma_start(out=outA.ap(), in_=evac[:])
    nc.compile()
    return nc


def _build_launch_b():
    nc = bacc.Bacc("TRN2", target_bir_lowering=False, debug=False)
    bf = mybir.dt.bfloat16
    f32 = mybir.dt.float32

    wev = nc.dram_tensor("wev", [CR, HT * P], bf, kind="ExternalInput")
    wod = nc.dram_tensor("wod", [CR, HT * P], bf, kind="ExternalInput")
    rtab = nc.dram_tensor("rtab", [CR, 4 * D], bf, kind="ExternalInput")
    pacc_d = nc.dram_tensor("pacc", [P, 4], f32, kind="ExternalOutput")

    AF = mybir.ActivationFunctionType

    with TileContext(nc) as tc:
        with (
            tc.tile_pool(name="sbuf", bufs=1) as pool,
            tc.tile_pool(name="work", bufs=3) as wpool,
            tc.tile_pool(name="psum", bufs=2, space="PSUM") as psum_pool,
        ):
            w_sb = pool.tile([P, HT, P], bf)
            rhs_sb = pool.tile([P, 4 * D], bf)
            dist = pool.tile([P, TC, 4], bf)
            sq = pool.tile([P, TC, 4], bf)
            pacc = pool.tile([P, 4], f32)

            nc.sync.dma_start(
                out=rhs_sb[0:CR, :], in_=rtab.ap()
            )
            nc.sync.dma_start(
                out=rhs_sb[64 : 64 + CR, :], in_=rtab.ap()
            )
            NCH = 6
            tch = HT // NCH
            for i in range(NCH):
                sl = slice(i * tch, (i + 1) * tch)
                nc.sync.dma_start(
                    out=w_sb[0:CR, sl, :],
                    in_=wev.ap().rearrange("r (t m) -> r t m", m=P)[:, sl, :],
                )
                nc.sync.dma_start(
                    out=w_sb[64 : 64 + CR, sl, :],
                    in_=wod.ap().rearrange("r (t m) -> r t m", m=P)[:, sl, :],
                )

            dist_h = dist[:].rearrange("p (h t) b -> p h t b", h=2)

            nwaves = HT // WT
            for w in range(nwaves):
                t0 = WT * w
                ps = psum_pool.tile(
                    [P, 2, WT, 4, D], mybir.dt.float32, tag="ps", name=f"ps_{w}"
                )
                for j in range(WT):
                    t = t0 + j
                    nc.tensor.matmul(
                        ps[:, 0, j, :, :].rearrange("p a b -> p (a b)"),
                        w_sb[0:CR, t, :],
                        rhs_sb[0:CR, :],
                        start=True,
                        stop=True,
                    )
                    nc.tensor.matmul(
                        ps[:, 1, j, :, :].rearrange("p a b -> p (a b)"),
                        w_sb[64 : 64 + CR, t, :],
                        rhs_sb[64 : 64 + CR, :],
                        start=True,
                        stop=True,
                    )
                kind = CONSUMER_PATTERN[w % len(CONSUMER_PATTERN)]
                out_ap = dist_h[:, :, t0 : t0 + WT, :]
                with nc.allow_low_precision("dist in bf16; error averages out"):
                    if kind == "D":
                        nc.vector.tensor_reduce(
                            out=out_ap,
                            in_=ps[:],
                            axis=mybir.AxisListType.X,
                            op=mybir.AluOpType.add,
                            apply_absolute_value=True,
                        )
                    else:
                        absd = wpool.tile([P, 2, WT, 4, D], bf, tag="absd")
                        nc.scalar.activation(out=absd[:], in_=ps[:], func=AF.Abs)
                        nc.vector.tensor_reduce(
                            out=out_ap,
                            in_=absd[:],
                            axis=mybir.AxisListType.X,
                            op=mybir.AluOpType.add,
                        )

            nc.vector.tensor_tensor(
                out=sq[:], in0=dist[:], in1=dist[:], op=mybir.AluOpType.mult
            )
            for b in range(4):
                nc.vector.tensor_reduce(
                    out=pacc[:, b : b + 1],
                    in_=sq[:, :, b],
                    axis=mybir.AxisListType.X,
                    op=mybir.AluOpType.add,
                )
            nc.sync.dma_start(out=pacc_d.ap(), in_=pacc[:])
    nc.compile()
    return nc


def _get(name):
    if name not in _built:
        if name == "A":
            _built[name] = _build_launch_a()
        else:
            _built[name] = _build_launch_b()
    return _built[name]


def _prep_a(emb_flat, lab_flat):
    in_maps = []
    for c in range(NCORES):
        e = emb_flat[c * NPIX : (c + 1) * NPIX].astype(BF16).reshape(P, TC, D)
        l = lab_flat[c * NPIX : (c + 1) * NPIX].reshape(P, TC)
        e17 = np.zeros((P, TCP, 17), dtype=BF16)
        e17[:, :TC, :D] = e
        e17[:, :, D] = BF16(1.0)
        w = np.zeros((P, WCOLS_PAD), dtype=BF16)
        w[:, : TCP * 17] = e17.reshape(P, TCP * 17)
        lb = np.full((P, TCP), LAB_PAD, dtype=BF16)
        lb[:, :TC] = l.astype(BF16)
        in_maps.append({"emb17": w, "labels": lb})
    return in_maps


def _reduce_a(results):
    sums = np.zeros((B, K, D), dtype=np.float64)
    cnts = np.zeros((B, K), dtype=np.float64)
    for c in range(NCORES):
        o = results[c]["outA"].astype(np.float64).reshape(P, NBANKS_A, 7, K)
        o = o.sum(axis=1)
        s = c // 2
        for j in range(7):
            blk = o[17 * j : 17 * j + 17, j, :]
            sums[s] += blk[:D].T
            cnts[s] += blk[D]
    cents = sums / np.maximum(cnts, 1.0)[:, :, None]
    cents = np.where(cnts[:, :, None] > 0, cents, 0.0)
    return cents, cnts


def _prep_b(emb_flat, lab_flat, cents):
    cb = cents.astype(F32)
    rtab = np.zeros((CR, 4 * D), dtype=BF16)
    rtab[:K, :] = cb.transpose(1, 0, 2).reshape(K, 4 * D).astype(BF16)
    eye = -np.eye(D, dtype=F32)
    for b in range(4):
        rtab[K:, b * D : (b + 1) * D] = eye.astype(BF16)

    in_maps = []
    kk = np.arange(K, dtype=np.int32)
    for c in range(NCORES):
        e = emb_flat[c * NPIX : (c + 1) * NPIX].astype(BF16).reshape(P, TC, D)
        l = lab_flat[c * NPIX : (c + 1) * NPIX].reshape(P, TC)
        oh = (l.T[:, None, :] == kk[None, :, None]).astype(BF16)
        eT = np.ascontiguousarray(e.transpose(1, 2, 0))
        w_all = np.concatenate([oh, eT], axis=1)
        wev = np.ascontiguousarray(w_all[:HT].transpose(1, 0, 2)).reshape(CR, HT * P)
        wod = np.ascontiguousarray(w_all[HT:].transpose(1, 0, 2)).reshape(CR, HT * P)
        in_maps.append({"wev": wev, "wod": wod, "rtab": rtab.copy()})
    return in_maps


def _push_host(cents):
    cb = cents.astype(np.float64)
    d = np.abs(cb[:, :, None, :] - cb[:, None, :, :]).sum(axis=-1)
    m = np.maximum(PUSH_MARGIN - d, 0.0)
    iu = np.triu(np.ones((K, K), dtype=bool), k=1)
    return (m * m * iu[None]).sum(axis=(1, 2)) / NCMP


def run_launches(embeddings, labels, trace=False, trace_kwargs=None):
    emb_flat = np.ascontiguousarray(np.asarray(embeddings), dtype=F32).reshape(
        NPIX_TOT, D
    )
    lab_flat = np.ascontiguousarray(np.asarray(labels), dtype=np.int32).reshape(
        NPIX_TOT
    )
    core_ids = list(range(NCORES))

    kwA = dict(trace=trace, **(trace_kwargs or {}))
    resA = run_bass_kernel_spmd(_get("A"), _prep_a(emb_flat, lab_flat), core_ids, **kwA)
    cents, _ = _reduce_a(resA.results)

    resB = run_bass_kernel_spmd(
        _get("B"), _prep_b(emb_flat, lab_flat, cents), core_ids, **kwA
    )
    pull = np.zeros(4, dtype=np.float64)
    for c in range(NCORES):
        pull += resB.results[c]["pacc"].astype(np.float64).sum(axis=0)
    pull /= NPIX_TOT

    push = _push_host(cents)

    loss = np.mean(PUSH_W * push + PULL_W * pull)
    return np.array(loss, dtype=F32), resA, resB


def kernel(embeddings, labels):
    loss, _, _ = run_launches(embeddings, labels, trace=False)
    return loss


# revision 12
# speedup vs baseline: 1.5793x; 1.1444x over previous
import numpy as np
import ml_dtypes

import concourse.bass as bass
import concourse.bacc as bacc
import concourse.mybir as mybir
from concourse.tile import TileContext
from concourse.bass_utils import run_bass_kernel_spmd

BF16 = ml_dtypes.bfloat16
F32 = np.float32

B, H, W, D, K = 4, 384, 384, 16, 32
NCORES = 8
NPIX_TOT = B * H * W
NPIX = NPIX_TOT // NCORES
P = 128
TC = NPIX // P
TCP = 588
NG = TCP // 7
GW = 7 * 17
WCOLS_PAD = 10016
NBANKS_A = 8
LAB_PAD = 100.0

HT = TC // 2
NW = HT // 8
CR = 48

PUSH_MARGIN = 0.25
PUSH_W = 1.0
PULL_W = 0.1
NCMP = K * (K - 1) / 2.0

CONSUMER_PATTERN = "AADADAADADAADADAAD"
WT = 16

_built = {}


def _build_launch_a():
    nc = bacc.Bacc("TRN2", target_bir_lowering=False, debug=False)
    bf = mybir.dt.bfloat16
    f32 = mybir.dt.float32

    emb17 = nc.dram_tensor("emb17", [P, WCOLS_PAD], bf, kind="ExternalInput")
    onehotA = nc.dram_tensor("onehotA", [P, TCP * K], bf, kind="ExternalInput")
    outA = nc.dram_tensor("outA", [P, NBANKS_A, 7 * K], f32, kind="ExternalOutput")

    with TileContext(nc) as tc:
        with (
            tc.tile_pool(name="sbuf", bufs=1) as pool,
            tc.tile_pool(name="psum", bufs=1, space="PSUM") as psum_pool,
        ):
            emb_sb = pool.tile([P, WCOLS_PAD], bf)
            onehot = pool.tile([P, TCP, K], bf)

            NCH = 4
            ch = WCOLS_PAD // NCH
            och = TCP // NCH
            for i in range(NCH):
                nc.sync.dma_start(
                    out=emb_sb[:, i * ch : (i + 1) * ch],
                    in_=emb17.ap()[:, i * ch : (i + 1) * ch],
                )
                nc.sync.dma_start(
                    out=onehot[:, i * och : (i + 1) * och, :],
                    in_=onehotA.ap().rearrange("p (t k) -> p t k", k=K)[
                        :, i * och : (i + 1) * och, :
                    ],
                )

            banks = [
                psum_pool.tile([P, 7, K], mybir.dt.float32, name=f"acc{b}")
                for b in range(NBANKS_A)
            ]
            for g in range(NG):
                nc.tensor.matmul(
                    banks[g % NBANKS_A][:],
                    emb_sb[:, GW * g : GW * g + 128],
                    onehot[:, 7 * g : 7 * g + 7, :],
                    start=(g < NBANKS_A),
                    stop=(g >= NG - NBANKS_A),
                )

            evac = pool.tile([P, NBANKS_A, 7 * K], f32)
            for b in range(NBANKS_A):
                eng = nc.scalar if b % 2 == 0 else nc.vector
                if b % 2 == 0:
                    nc.scalar.copy(
                        out=evac[:, b, :],
                        in_=banks[b][:].rearrange("p a b -> p (a b)"),
                    )
                else:
                    nc.vector.tensor_copy(
                        out=evac[:, b, :],
                        in_=banks[b][:].rearrange("p a b -> p (a b)"),
                    )
            nc.sync.dma_start(out=outA.ap(), in_=evac[:])
    nc.compile()
    return nc


def _build_launch_b():
    nc = bacc.Bacc("TRN2", target_bir_lowering=False, debug=False)
    bf = mybir.dt.bfloat16
    f32 = mybir.dt.float32

    wev = nc.dram_tensor("wev", [CR, HT * P], bf, kind="ExternalInput")
    wod = nc.dram_tensor("wod", [CR, HT * P], bf, kind="ExternalInput")
    rtab = nc.dram_tensor("rtab", [CR, 4 * D], bf, kind="ExternalInput")
    pacc_d = nc.dram_tensor("pacc", [P, 4], f32, kind="ExternalOutput")

    AF = mybir.ActivationFunctionType

    with TileContext(nc) as tc:
        with (
            tc.tile_pool(name="sbuf", bufs=1) as pool,
            tc.tile_pool(name="work", bufs=3) as wpool,
            tc.tile_pool(name="psum", bufs=2, space="PSUM") as psum_pool,
        ):
            w_sb = pool.tile([P, HT, P], bf)
            rhs_sb = pool.tile([P, 4 * D], bf)
            dist = pool.tile([P, TC, 4], bf)
            sq = pool.tile([P, TC, 4], bf)
            pacc = pool.tile([P, 4], f32)

            nc.sync.dma_start(
                out=rhs_sb[0:CR, :], in_=rtab.ap()
            )
            nc.sync.dma_start(
                out=rhs_sb[64 : 64 + CR, :], in_=rtab.ap()
            )
            NCH = 6
            tch = HT // NCH
            for i in range(NCH):
                sl = slice(i * tch, (i + 1) * tch)
                nc.sync.dma_start(
                    out=w_sb[0:CR, sl, :],
                    in_=wev.ap().rearrange("r (t m) -> r t m", m=P)[:, sl, :],
                )
                nc.sync.dma_start(
                    out=w_sb[64 : 64 + CR, sl, :],
                    in_=wod.ap().rearrange("r (t m) -> r t m", m=P)[:, sl, :],
                )

            nwaves = HT // WT
            for w in range(nwaves):
                t0 = WT * w
                ps = psum_pool.tile(
                    [P, 2, WT, 4, D], mybir.dt.float32, tag="ps", name=f"ps_{w}"
                )
                for j in range(WT):
                    t = t0 + j
                    nc.tensor.matmul(
                        ps[:, 0, j, :, :].rearrange("p a b -> p (a b)"),
                        w_sb[0:CR, t, :],
                        rhs_sb[0:CR, :],
                        start=True,
                        stop=True,
                    )
                    nc.tensor.matmul(
                        ps[:, 1, j, :, :].rearrange("p a b -> p (a b)"),
                        w_sb[64 : 64 + CR, t, :],
                        rhs_sb[64 : 64 + CR, :],
                        start=True,
                        stop=True,
                    )
                kind = CONSUMER_PATTERN[w % len(CONSUMER_PATTERN)]
                out_ap = dist[:, 2 * t0 : 2 * t0 + 2 * WT, :].rearrange(
                    "p (h t) b -> p h t b", h=2
                )
                with nc.allow_low_precision("dist in bf16; error averages out"):
                    if kind == "D":
                        nc.vector.tensor_reduce(
                            out=out_ap,
                            in_=ps[:],
                            axis=mybir.AxisListType.X,
                            op=mybir.AluOpType.add,
                            apply_absolute_value=True,
                        )
                    else:
                        absd = wpool.tile([P, 2, WT, 4, D], bf, tag="absd")
                        nc.scalar.activation(out=absd[:], in_=ps[:], func=AF.Abs)
                        nc.vector.tensor_reduce(
                            out=out_ap,
                            in_=absd[:],
                            axis=mybir.AxisListType.X,
                            op=mybir.AluOpType.add,
                        )

            nc.vector.tensor_tensor(
                out=sq[:], in0=dist[:], in1=dist[:], op=mybir.AluOpType.mult
            )
            for b in range(4):
                nc.vector.tensor_reduce(
                    out=pacc[:, b : b + 1],
                    in_=sq[:, :, b],
                    axis=mybir.AxisListType.X,
                    op=mybir.AluOpType.add,
                )
            nc.sync.dma_start(out=pacc_d.ap(), in_=pacc[:])
    nc.compile()
    return nc


def _get(name):
    if name not in _built:
        if name == "A":
            _built[name] = _build_launch_a()
        else:
            _built[name] = _build_launch_b()
    return _built[name]


def _prep_a(emb_flat, lab_flat):
    in_maps = []
    kk = np.arange(K, dtype=np.int32)
    for c in range(NCORES):
        e = emb_flat[c * NPIX : (c + 1) * NPIX].astype(BF16).reshape(P, TC, D)
        l = lab_flat[c * NPIX : (c + 1) * NPIX].reshape(P, TC)
        e17 = np.zeros((P, TCP, 17), dtype=BF16)
        e17[:, :TC, :D] = e
        e17[:, :, D] = BF16(1.0)
        w = np.zeros((P, WCOLS_PAD), dtype=BF16)
        w[:, : TCP * 17] = e17.reshape(P, TCP * 17)
        oh = np.zeros((P, TCP, K), dtype=BF16)
        oh[:, :TC, :] = (l[:, :, None] == kk[None, None, :]).astype(BF16)
        in_maps.append({"emb17": w, "onehotA": oh.reshape(P, TCP * K)})
    return in_maps


def _reduce_a(results):
    sums = np.zeros((B, K, D), dtype=np.float64)
    cnts = np.zeros((B, K), dtype=np.float64)
    for c in range(NCORES):
        o = results[c]["outA"].astype(np.float64).reshape(P, NBANKS_A, 7, K)
        o = o.sum(axis=1)
        s = c // 2
        for j in range(7):
            blk = o[17 * j : 17 * j + 17, j, :]
            sums[s] += blk[:D].T
            cnts[s] += blk[D]
    cents = sums / np.maximum(cnts, 1.0)[:, :, None]
    cents = np.where(cnts[:, :, None] > 0, cents, 0.0)
    return cents, cnts


def _prep_b(emb_flat, lab_flat, cents):
    cb = cents.astype(F32)
    rtab = np.zeros((CR, 4 * D), dtype=BF16)
    rtab[:K, :] = cb.transpose(1, 0, 2).reshape(K, 4 * D).astype(BF16)
    eye = -np.eye(D, dtype=F32)
    for b in range(4):
        rtab[K:, b * D : (b + 1) * D] = eye.astype(BF16)

    in_maps = []
    kk = np.arange(K, dtype=np.int32)
    for c in range(NCORES):
        e = emb_flat[c * NPIX : (c + 1) * NPIX].astype(BF16).reshape(P, TC, D)
        l = lab_flat[c * NPIX : (c + 1) * NPIX].reshape(P, TC)
        oh = (l.T[:, None, :] == kk[None, :, None]).astype(BF16)
        eT = np.ascontiguousarray(e.transpose(1, 2, 0))
        w_all = np.concatenate([oh, eT], axis=1)
        wev = np.ascontiguousarray(w_all[:HT].transpose(1, 0, 2)).reshape(CR, HT * P)
        wod = np.ascontiguousarray(w_all[HT:].transpose(1, 0, 2)).reshape(CR, HT * P)
        in_maps.append({"wev": wev, "wod": wod, "rtab": rtab.copy()})
    return in_maps


def _push_host(cents):
    cb = cents.astype(np.float64)
    d = np.abs(cb[:, :, None, :] - cb[:, None, :, :]).sum(axis=-1)
    m = np.maximum(PUSH_MARGIN - d, 0.0)
    iu = np.triu(np.ones((K, K), dtype=bool), k=1)
    return (m * m * iu[None]).sum(axis=(1, 2)) / NCMP


def run_launches(embeddings, labels, trace=False, trace_kwargs=None):
    emb_flat = np.ascontiguousarray(np.asarray(embeddings), dtype=F32).reshape(
        NPIX_TOT, D
    )
    lab_flat = np.ascontiguousarray(np.asarray(labels), dtype=np.int32).reshape(
        NPIX_TOT
    )
    core_ids = list(range(NCORES))

    kwA = dict(trace=trace, **(trace_kwargs or {}))
    resA = run_bass_kernel_spmd(_get("A"), _prep_a(emb_flat, lab_flat), core_ids, **kwA)
    cents, _ = _reduce_a(resA.results)

    resB = run_bass_kernel_spmd(
        _get("B"), _prep_b(emb_flat, lab_flat, cents), core_ids, **kwA
    )
    pull = np.zeros(4, dtype=np.float64)
    for c in range(NCORES):
        pull += resB.results[c]["pacc"].astype(np.float64).sum(axis=0)
    pull /= NPIX_TOT

    push = _push_host(cents)

    loss = np.mean(PUSH_W * push + PULL_W * pull)
    return np.array(loss, dtype=F32), resA, resB


def kernel(embeddings, labels):
    loss, _, _ = run_launches(embeddings, labels, trace=False)
    return loss


# revision 14
# speedup vs baseline: 1.6245x; 1.0286x over previous
import numpy as np
import ml_dtypes

import concourse.bass as bass
import concourse.bacc as bacc
import concourse.mybir as mybir
from concourse.tile import TileContext
from concourse.bass_utils import run_bass_kernel_spmd

BF16 = ml_dtypes.bfloat16
F32 = np.float32

B, H, W, D, K = 4, 384, 384, 16, 32
NCORES = 8
NPIX_TOT = B * H * W
NPIX = NPIX_TOT // NCORES
P = 128
TC = NPIX // P
TCP = 588
NG = TCP // 7
GW = 7 * 17
WCOLS_PAD = 10016
NBANKS_A = 8
LAB_PAD = 100.0

HT = TC // 2
NW = HT // 8
CR = 48

PUSH_MARGIN = 0.25
PUSH_W = 1.0
PULL_W = 0.1
NCMP = K * (K - 1) / 2.0

CONSUMER_PATTERN = "AADAAAAAAADAAAAAAA"
WT = 16

_built = {}


def _build_launch_a():
    nc = bacc.Bacc("TRN2", target_bir_lowering=False, debug=False)
    bf = mybir.dt.bfloat16
    f32 = mybir.dt.float32

    emb17 = nc.dram_tensor("emb17", [P, WCOLS_PAD], bf, kind="ExternalInput")
    onehotA = nc.dram_tensor("onehotA", [P, TCP * K], bf, kind="ExternalInput")
    outA = nc.dram_tensor("outA", [P, NBANKS_A, 7 * K], f32, kind="ExternalOutput")

    with TileContext(nc) as tc:
        with (
            tc.tile_pool(name="sbuf", bufs=1) as pool,
            tc.tile_pool(name="psum", bufs=1, space="PSUM") as psum_pool,
        ):
            emb_sb = pool.tile([P, WCOLS_PAD], bf)
            onehot = pool.tile([P, TCP, K], bf)

            NCH = 4
            ch = WCOLS_PAD // NCH
            och = TCP // NCH
            for i in range(NCH):
                nc.sync.dma_start(
                    out=emb_sb[:, i * ch : (i + 1) * ch],
                    in_=emb17.ap()[:, i * ch : (i + 1) * ch],
                )
                nc.sync.dma_start(
                    out=onehot[:, i * och : (i + 1) * och, :],
                    in_=onehotA.ap().rearrange("p (t k) -> p t k", k=K)[
                        :, i * och : (i + 1) * och, :
                    ],
                )

            banks = [
                psum_pool.tile([P, 7, K], mybir.dt.float32, name=f"acc{b}")
                for b in range(NBANKS_A)
            ]
            for g in range(NG):
                nc.tensor.matmul(
                    banks[g % NBANKS_A][:],
                    emb_sb[:, GW * g : GW * g + 128],
                    onehot[:, 7 * g : 7 * g + 7, :],
                    start=(g < NBANKS_A),
                    stop=(g >= NG - NBANKS_A),
                )

            evac = pool.tile([P, NBANKS_A, 7 * K], f32)
            for b in range(NBANKS_A):
                eng = nc.scalar if b % 2 == 0 else nc.vector
                if b % 2 == 0:
                    nc.scalar.copy(
                        out=evac[:, b, :],
                        in_=banks[b][:].rearrange("p a b -> p (a b)"),
                    )
                else:
                    nc.vector.tensor_copy(
                        out=evac[:, b, :],
                        in_=banks[b][:].rearrange("p a b -> p (a b)"),
                    )
            nc.sync.dma_start(out=outA.ap(), in_=evac[:])
    nc.compile()
    return nc


def _build_launch_b():
    nc = bacc.Bacc("TRN2", target_bir_lowering=False, debug=False)
    bf = mybir.dt.bfloat16
    f32 = mybir.dt.float32

    wev = nc.dram_tensor("wev", [CR, HT * P], bf, kind="ExternalInput")
    wod = nc.dram_tensor("wod", [CR, HT * P], bf, kind="ExternalInput")
    rtab = nc.dram_tensor("rtab", [CR, 4 * D], bf, kind="ExternalInput")
    pacc_d = nc.dram_tensor("pacc", [P, 4], f32, kind="ExternalOutput")

    AF = mybir.ActivationFunctionType

    with TileContext(nc) as tc:
        with (
            tc.tile_pool(name="sbuf", bufs=1) as pool,
            tc.tile_pool(name="work", bufs=3) as wpool,
            tc.tile_pool(name="psum", bufs=2, space="PSUM") as psum_pool,
        ):
            w_sb = pool.tile([P, HT, P], bf)
            rhs_sb = pool.tile([P, 4 * D], bf)
            dist = pool.tile([P, TC, 4], bf)
            sq = pool.tile([P, TC, 4], bf)
            pacc = pool.tile([P, 4], f32)

            nc.sync.dma_start(
                out=rhs_sb[0:CR, :], in_=rtab.ap()
            )
            nc.sync.dma_start(
                out=rhs_sb[64 : 64 + CR, :], in_=rtab.ap()
            )
            NCH = 6
            tch = HT // NCH
            for i in range(NCH):
                sl = slice(i * tch, (i + 1) * tch)
                nc.sync.dma_start(
                    out=w_sb[0:CR, sl, :],
                    in_=wev.ap().rearrange("r (t m) -> r t m", m=P)[:, sl, :],
                )
                nc.sync.dma_start(
                    out=w_sb[64 : 64 + CR, sl, :],
                    in_=wod.ap().rearrange("r (t m) -> r t m", m=P)[:, sl, :],
                )

            nwaves = HT // WT
            for w in range(nwaves):
                t0 = WT * w
                ps = psum_pool.tile(
                    [P, 2, WT, 4, D], mybir.dt.float32, tag="ps", name=f"ps_{w}"
                )
                for j in range(WT):
                    t = t0 + j
                    nc.tensor.matmul(
                        ps[:, 0, j, :, :].rearrange("p a b -> p (a b)"),
                        w_sb[0:CR, t, :],
                        rhs_sb[0:CR, :],
                        start=True,
                        stop=True,
                    )
                    nc.tensor.matmul(
                        ps[:, 1, j, :, :].rearrange("p a b -> p (a b)"),
                        w_sb[64 : 64 + CR, t, :],
                        rhs_sb[64 : 64 + CR, :],
                        start=True,
                        stop=True,
                    )
                kind = CONSUMER_PATTERN[w % len(CONSUMER_PATTERN)]
                out_ap = dist[:, 2 * t0 : 2 * t0 + 2 * WT, :].rearrange(
                    "p (h t) b -> p h t b", h=2
                )
                with nc.allow_low_precision("dist in bf16; error averages out"):
                    if kind == "D":
                        nc.vector.tensor_reduce(
                            out=out_ap,
                            in_=ps[:],
                            axis=mybir.AxisListType.X,
                            op=mybir.AluOpType.add,
                            apply_absolute_value=True,
                        )
                    else:
                        absd = wpool.tile([P, 2, WT, 4, D], bf, tag="absd")
                        nc.scalar.activation(out=absd[:], in_=ps[:], func=AF.Abs)
                        h1 = wpool.tile([P, 2, WT, 4, D // 2], bf, tag="h1")
                        nc.vector.tensor_tensor(
                            out=h1[:],
                            in0=absd[:, :, :, :, 0 : D // 2],
                            in1=absd[:, :, :, :, D // 2 : D],
                            op=mybir.AluOpType.add,
                        )
                        h2 = wpool.tile([P, 2, WT, 4, D // 4], bf, tag="h2")
                        nc.vector.tensor_tensor(
                            out=h2[:],
                            in0=h1[:, :, :, :, 0 : D // 4],
                            in1=h1[:, :, :, :, D // 4 : D // 2],
                            op=mybir.AluOpType.add,
                        )
                        nc.vector.tensor_reduce(
                            out=out_ap,
                            in_=h2[:],
                            axis=mybir.AxisListType.X,
                            op=mybir.AluOpType.add,
                        )

            nc.vector.tensor_tensor(
                out=sq[:], in0=dist[:], in1=dist[:], op=mybir.AluOpType.mult
            )
            for b in range(4):
                nc.vector.tensor_reduce(
                    out=pacc[:, b : b + 1],
                    in_=sq[:, :, b],
                    axis=mybir.AxisListType.X,
                    op=mybir.AluOpType.add,
                )
            nc.sync.dma_start(out=pacc_d.ap(), in_=pacc[:])
    nc.compile()
    return nc


def _get(name):
    if name not in _built:
        if name == "A":
            _built[name] = _build_launch_a()
        else:
            _built[name] = _build_launch_b()
    return _built[name]


def _prep_a(emb_flat, lab_flat):
    in_maps = []
    kk = np.arange(K, dtype=np.int32)
    for c in range(NCORES):
        e = emb_flat[c * NPIX : (c + 1) * NPIX].astype(BF16).reshape(P, TC, D)
        l = lab_flat[c * NPIX : (c + 1) * NPIX].reshape(P, TC)
        e17 = np.zeros((P, TCP, 17), dtype=BF16)
        e17[:, :TC, :D] = e
        e17[:, :, D] = BF16(1.0)
        w = np.zeros((P, WCOLS_PAD), dtype=BF16)
        w[:, : TCP * 17] = e17.reshape(P, TCP * 17)
        oh = np.zeros((P, TCP, K), dtype=BF16)
        oh[:, :TC, :] = (l[:, :, None] == kk[None, None, :]).astype(BF16)
        in_maps.append({"emb17": w, "onehotA": oh.reshape(P, TCP * K)})
    return in_maps


def _reduce_a(results):
    sums = np.zeros((B, K, D), dtype=np.float64)
    cnts = np.zeros((B, K), dtype=np.float64)
    for c in range(NCORES):
        o = results[c]["outA"].astype(np.float64).reshape(P, NBANKS_A, 7, K)
        o = o.sum(axis=1)
        s = c // 2
        for j in range(7):
            blk = o[17 * j : 17 * j + 17, j, :]
            sums[s] += blk[:D].T
            cnts[s] += blk[D]
    cents = sums / np.maximum(cnts, 1.0)[:, :, None]
    cents = np.where(cnts[:, :, None] > 0, cents, 0.0)
    return cents, cnts


def _prep_b(emb_flat, lab_flat, cents):
    cb = cents.astype(F32)
    rtab = np.zeros((CR, 4 * D), dtype=BF16)
    rtab[:K, :] = cb.transpose(1, 0, 2).reshape(K, 4 * D).astype(BF16)
    eye = -np.eye(D, dtype=F32)
    for b in range(4):
        rtab[K:, b * D : (b + 1) * D] = eye.astype(BF16)

    in_maps = []
    kk = np.arange(K, dtype=np.int32)
    for c in range(NCORES):
        e = emb_flat[c * NPIX : (c + 1) * NPIX].astype(BF16).reshape(P, TC, D)
        l = lab_flat[c * NPIX : (c + 1) * NPIX].reshape(P, TC)
        oh = (l.T[:, None, :] == kk[None, :, None]).astype(BF16)
        eT = np.ascontiguousarray(e.transpose(1, 2, 0))
        w_all = np.concatenate([oh, eT], axis=1)
        wev = np.ascontiguousarray(w_all[:HT].transpose(1, 0, 2)).reshape(CR, HT * P)
        wod = np.ascontiguousarray(w_all[HT:].transpose(1, 0, 2)).reshape(CR, HT * P)
        in_maps.append({"wev": wev, "wod": wod, "rtab": rtab.copy()})
    return in_maps


def _push_host(cents):
    cb = cents.astype(np.float64)
    d = np.abs(cb[:, :, None, :] - cb[:, None, :, :]).sum(axis=-1)
    m = np.maximum(PUSH_MARGIN - d, 0.0)
    iu = np.triu(np.ones((K, K), dtype=bool), k=1)
    return (m * m * iu[None]).sum(axis=(1, 2)) / NCMP


def run_launches(embeddings, labels, trace=False, trace_kwargs=None):
    emb_flat = np.ascontiguousarray(np.asarray(embeddings), dtype=F32).reshape(
        NPIX_TOT, D
    )
    lab_flat = np.ascontiguousarray(np.asarray(labels), dtype=np.int32).reshape(
        NPIX_TOT
    )
    core_ids = list(range(NCORES))

    kwA = dict(trace=trace, **(trace_kwargs or {}))
    resA = run_bass_kernel_spmd(_get("A"), _prep_a(emb_flat, lab_flat), core_ids, **kwA)
    cents, _ = _reduce_a(resA.results)

    resB = run_bass_kernel_spmd(
        _get("B"), _prep_b(emb_flat, lab_flat, cents), core_ids, **kwA
    )
    pull = np.zeros(4, dtype=np.float64)
    for c in range(NCORES):
        pull += resB.results[c]["pacc"].astype(np.float64).sum(axis=0)
    pull /= NPIX_TOT

    push = _push_host(cents)

    loss = np.mean(PUSH_W * push + PULL_W * pull)
    return np.array(loss, dtype=F32), resA, resB


def kernel(embeddings, labels):
    loss, _, _ = run_launches(embeddings, labels, trace=False)
    return loss
